# revision 14
# baseline (speedup 1.0000x reference)
"""Trainium2 Bass kernel for nn_ContrastiveNoAugLoss.

loss = mean((x_emd - (max(z_cos) - z_cos))^2) where
  x_emd[i,j] = mean_n |sorted(x_i)[n] - sorted(x_j)[n]|   (1D Wasserstein)
  z_cos = zn @ zn.T with zn = z / max(||z_i||, eps)

Device strategy (8 cores, data-parallel over the i-axis of the [B,B] pair
matrix; each core owns 16 rows and computes M[:, k] = sum_n max(xs_j, xs_myk)
for all 128 j with one fused scalar_tensor_tensor(max, accum) DVE
instruction per row, using sum|a-b| = 2*sum max(a,b) - sum a - sum b).
The z cosine matrix, its global max m, and the per-core partial sums
sum(t), sum(t^2) of t = x_emd + z_cos are all computed on-device; the host
combines 8 partial scalars:
  loss*B^2 = sum(t^2) - 2*m*sum(t) + B^2*m^2.

Host does only O(B*N log N) prep: sort, bf16 cast, transpose-free packing,
row sums, z norms.
"""
import numpy as np
import ml_dtypes

import concourse.bass as bass
from concourse import bacc
import concourse.mybir as mybir
from concourse import bass_isa
from concourse.tile import TileContext
from concourse.bass_utils import run_bass_kernel_spmd

B = 128          # batch (pair-matrix side)
N = 3072         # samples per row (3*32*32)
D = 128          # z embedding dim
NCORES = 8
RPC = B // NCORES  # rows per core = 16
EPS = 1e-12

_BF16 = mybir.dt.bfloat16
_F32 = mybir.dt.float32

_cached_nc = None


def _build_nc():
    nc = bacc.Bacc(
        "TRN2",
        target_bir_lowering=False,
        debug=False,
        enable_asserts=True,
        num_devices=NCORES,
    )

    xs_d = nc.dram_tensor("xs", [B, N], _BF16, kind="ExternalInput")
    rows_d = nc.dram_tensor("rows", [RPC, N], _BF16, kind="ExternalInput")
    zt_d = nc.dram_tensor("zt", [D, B], _F32, kind="ExternalInput")
    ztmy_d = nc.dram_tensor("ztmy", [D, RPC], _F32, kind="ExternalInput")
    rmy_d = nc.dram_tensor("rmy", [B, RPC], _F32, kind="ExternalInput")
    sbmy_d = nc.dram_tensor("sbmy", [B, RPC], _F32, kind="ExternalInput")
    rfull_d = nc.dram_tensor("rfull", [B, B], _F32, kind="ExternalInput")
    out_d = nc.dram_tensor("out", [1, 8], _F32, kind="ExternalOutput")

    with TileContext(nc) as tc:
        with tc.tile_pool(name="big", bufs=1) as big, tc.tile_pool(
            name="bc", bufs=4
        ) as bcp, tc.tile_pool(name="scr", bufs=2) as scrp, tc.tile_pool(
            name="small", bufs=1
        ) as sm, tc.tile_pool(name="ps", bufs=1, space="PSUM") as pps:
            xs_sb = big.tile([B, N], _BF16)
            nc.sync.dma_start(xs_sb, xs_d.ap())

            zt_sb = sm.tile([D, B], _F32)
            nc.sync.dma_start(zt_sb, zt_d.ap())
            ztmy_sb = sm.tile([D, RPC], _F32)
            nc.sync.dma_start(ztmy_sb, ztmy_d.ap())
            rmy_sb = sm.tile([B, RPC], _F32)
            nc.sync.dma_start(rmy_sb, rmy_d.ap())
            sbmy_sb = sm.tile([B, RPC], _F32)
            nc.sync.dma_start(sbmy_sb, sbmy_d.ap())
            rfull_sb = sm.tile([B, B], _F32)
            nc.sync.dma_start(rfull_sb, rfull_d.ap())

            # ---- main loop: M[:, k] = sum_n max(xs[j, n], rows[k, n]) ----
            mcols = sm.tile([B, RPC], _F32)
            for k in range(RPC):
                bc = bcp.tile([B, N], _BF16, tag="bc")
                nc.sync.dma_start(bc, rows_d.ap()[k : k + 1, :].broadcast_to((B, N)))
                scratch = scrp.tile([B, N], _BF16, tag="scratch")
                nc.vector.scalar_tensor_tensor(
                    out=scratch,
                    in0=xs_sb,
                    scalar=1.0,
                    in1=bc,
                    op0=mybir.AluOpType.mult,
                    op1=mybir.AluOpType.max,
                    accum_out=mcols[:, k : k + 1],
                )

            # ---- z side (overlaps on PE/ACT/GPSIMD) ----
            g_ps = pps.tile([B, RPC], _F32)
            nc.tensor.matmul(g_ps, zt_sb, ztmy_sb, start=True, stop=True)
            gf_ps = pps.tile([B, B], _F32)
            nc.tensor.matmul(gf_ps, zt_sb, zt_sb, start=True, stop=True)

            # Small-AP instructions lower to compact ISA structs that fit a
            # single sem wait, so pre-consume every cross-engine dependency
            # with one TS-struct copy each (all on DVE, program order);
            # the real tail ops then only carry same-engine deps.
            def ts_copy(dst, src):
                nc.vector.tensor_scalar(
                    out=dst, in0=src, scalar1=1.0, scalar2=None,
                    op0=mybir.AluOpType.mult,
                )

            gf_sb = sm.tile([B, B], _F32)
            ts_copy(gf_sb, gf_ps)
            g_sb = sm.tile([B, RPC], _F32)
            ts_copy(g_sb, g_ps)
            rfullc = sm.tile([B, B], _F32)
            ts_copy(rfullc, rfull_sb)
            rmyc = sm.tile([B, RPC], _F32)
            ts_copy(rmyc, rmy_sb)
            sbmyc = sm.tile([B, RPC], _F32)
            ts_copy(sbmyc, sbmy_sb)

            zcf = sm.tile([B, B], _F32)
            nc.vector.scalar_tensor_tensor(
                out=zcf,
                in0=gf_sb,
                scalar=1.0,
                in1=rfullc,
                op0=mybir.AluOpType.mult,
                op1=mybir.AluOpType.mult,
            )
            mx = sm.tile([B, 1], _F32)
            nc.vector.tensor_reduce(
                mx, zcf, mybir.AxisListType.X, mybir.AluOpType.max
            )
            mxa = sm.tile([B, 1], _F32)
            nc.gpsimd.partition_all_reduce(mxa, mx, B, bass_isa.ReduceOp.max)

            zc = sm.tile([B, RPC], _F32)
            nc.vector.scalar_tensor_tensor(
                out=zc,
                in0=g_sb,
                scalar=1.0,
                in1=rmyc,
                op0=mybir.AluOpType.mult,
                op1=mybir.AluOpType.mult,
            )

            # ---- t = (2/N)*M - (S_j+S_myk)/N + zcos ----
            t1 = sm.tile([B, RPC], _F32)
            nc.vector.scalar_tensor_tensor(
                out=t1,
                in0=mcols,
                scalar=2.0 / N,
                in1=sbmyc,
                op0=mybir.AluOpType.mult,
                op1=mybir.AluOpType.subtract,
            )
            t = sm.tile([B, RPC], _F32)
            junk1 = sm.tile([B, RPC], _F32)
            q1c = sm.tile([B, 1], _F32)
            # t = t1 + zc, and accumulate q1 = sum_k t in the same pass
            nc.vector.scalar_tensor_tensor(
                out=t,
                in0=t1,
                scalar=0.0,
                in1=zc,
                op0=mybir.AluOpType.add,
                op1=mybir.AluOpType.add,
                accum_out=q1c,
            )
            q2c = sm.tile([B, 1], _F32)
            nc.vector.scalar_tensor_tensor(
                out=junk1,
                in0=t,
                scalar=1.0,
                in1=t,
                op0=mybir.AluOpType.mult,
                op1=mybir.AluOpType.mult,
                accum_out=q2c,
            )
            q1a = sm.tile([B, 1], _F32)
            nc.gpsimd.partition_all_reduce(q1a, q1c, B, bass_isa.ReduceOp.add)
            q2a = sm.tile([B, 1], _F32)
            nc.gpsimd.partition_all_reduce(q2a, q2c, B, bass_isa.ReduceOp.add)

            out_sb = sm.tile([1, 8], _F32)
            nc.gpsimd.memset(out_sb, 0.0)
            nc.scalar.copy(out_sb[0:1, 0:1], q2a[0:1, 0:1])
            nc.scalar.copy(out_sb[0:1, 1:2], q1a[0:1, 0:1])
            nc.scalar.copy(out_sb[0:1, 2:3], mxa[0:1, 0:1])
            nc.sync.dma_start(out_d.ap(), out_sb)
    return nc


def _get_nc():
    global _cached_nc
    if _cached_nc is None:
        _cached_nc = _build_nc()
        _cached_nc.finalize()
    return _cached_nc


def _prep_inputs(z, x):
    z = np.asarray(z, dtype=np.float32).reshape(B, D)
    x = np.asarray(x, dtype=np.float32).reshape(B, N)

    xs = np.sort(x, axis=1)
    xb = xs.astype(ml_dtypes.bfloat16)
    S = xb.astype(np.float64).sum(axis=1)  # row sums of the bf16 values

    norms = np.sqrt((z.astype(np.float64) ** 2).sum(axis=1))
    r = 1.0 / np.maximum(norms, EPS)

    zt = np.ascontiguousarray(z.T)  # [D, B] f32
    rfull = np.ascontiguousarray(np.outer(r, r).astype(np.float32))

    in_maps = []
    for c in range(NCORES):
        my = slice(c * RPC, (c + 1) * RPC)
        in_maps.append(
            {
                "xs": xb,
                "rows": np.ascontiguousarray(xb[my]),
                "zt": zt,
                "ztmy": np.ascontiguousarray(zt[:, my]),
                "rmy": np.ascontiguousarray(np.outer(r, r[my]).astype(np.float32)),
                "sbmy": np.ascontiguousarray(
                    ((S[:, None] + S[None, my]) / float(N)).astype(np.float32)
                ),
                "rfull": rfull,
            }
        )
    return in_maps


def _combine(results):
    T2 = 0.0
    T1 = 0.0
    for res in results:
        o = np.asarray(res["out"], dtype=np.float64).reshape(-1)
        T2 += o[0]
        T1 += o[1]
    m = float(np.asarray(results[0]["out"], dtype=np.float64).reshape(-1)[2])
    bsq = float(B * B)
    loss = (T2 - 2.0 * m * T1 + bsq * m * m) / bsq
    return np.float32(loss)


def run_device(z, x, **kwargs):
    """Run the SPMD bass kernel; kwargs forwarded (e.g. trace=True)."""
    nc = _get_nc()
    in_maps = _prep_inputs(z, x)
    res = run_bass_kernel_spmd(nc, in_maps, core_ids=list(range(NCORES)), **kwargs)
    return res


def kernel(z, x):
    res = run_device(z, x)
    return _combine(res.results)
